# revision 36
# baseline (speedup 1.0000x reference)
"""Trainium2 Bass kernel for nn_MultiHeadAttention_77446850281793.

Reference semantics (faithful quirk: softmax over the HEADS axis):
    Qh = q @ Wq.T + bq   (per-head view)   [S, H, dk]
    scores[h, i, j] = (Qh[i,h] . Kh[j,h]) / sqrt(dk)
    attn = softmax over h (heads) of scores
    ctx[h, i] = sum_j attn[h,i,j] * Vh[j,h]
    out = concat(ctx) @ Wo.T + bo

Sharding: sequence-parallel over the 8 cores (256 query rows each).
Each core projects its own 256-row slice of q/k/v; K^T and V slices are
AllGathered (bf16) so every core holds full K/V; the head-axis softmax is
then entirely core-local. Output rows are gathered on the host.
"""

import numpy as np
import ml_dtypes

SEQ, DIM, HEADS, DK, NCORES = 2048, 1024, 16, 64, 8
SS = SEQ // NCORES  # 256 query rows per core
SCALE = 1.0 / 8.0  # 1/sqrt(DK); folded into Wq/bq on the host

_CACHE = {}


def _build(fake_ag=False):
    import concourse.bass as bass
    import concourse.bacc as bacc
    import concourse.tile as tile
    import concourse.mybir as mybir

    dt = mybir.dt
    f32, bf16 = dt.float32, dt.bfloat16
    AF = mybir.ActivationFunctionType

    nc = bacc.Bacc(
        "TRN2", target_bir_lowering=False, debug=False, num_devices=NCORES
    )

    qT = nc.dram_tensor("qT", [DIM, SS], bf16, kind="ExternalInput")
    kT = nc.dram_tensor("kT", [DIM, SS], bf16, kind="ExternalInput")
    vT = nc.dram_tensor("vT", [DIM, SS], bf16, kind="ExternalInput")
    WqT = nc.dram_tensor("WqT", [DIM, DIM], bf16, kind="ExternalInput")
    WkT = nc.dram_tensor("WkT", [DIM, DIM], bf16, kind="ExternalInput")
    WvT = nc.dram_tensor("WvT", [DIM, DIM], bf16, kind="ExternalInput")
    WoT = nc.dram_tensor("WoT", [DIM, DIM], f32, kind="ExternalInput")
    bq = nc.dram_tensor("bq", [DIM], f32, kind="ExternalInput")
    bk = nc.dram_tensor("bk", [DIM], f32, kind="ExternalInput")
    bv = nc.dram_tensor("bv", [DIM], f32, kind="ExternalInput")
    bo = nc.dram_tensor("bo", [DIM], f32, kind="ExternalInput")
    out = nc.dram_tensor("out", [SS, DIM], f32, kind="ExternalOutput")

    with tile.TileContext(nc) as tc:
        _emit(nc, tc, bass, mybir, locals(), fake_ag=fake_ag)
    nc.compile()
    return nc


def _emit(nc, tc, bass, mybir, io, fake_ag=False):
    dt = mybir.dt
    f32, bf16 = dt.float32, dt.bfloat16
    AF = mybir.ActivationFunctionType
    qT, kT, vT = io["qT"], io["kT"], io["vT"]
    WqT, WkT, WvT, WoT = io["WqT"], io["WkT"], io["WvT"], io["WoT"]
    bq, bk, bv, bo = io["bq"], io["bk"], io["bv"], io["bo"]
    out = io["out"]

    # head h -> column slot in the per-j-tile score/exp buffers. Scores are
    # computed in groups of 4 heads (one 2-bank PSUM tile per group, double
    # buffered); the two heads of a concurrent row-packed matmul pair are
    # placed in different PSUM banks.
    def slot_col(h):
        g, u, par = h // 4, (h % 4) // 2, h % 2
        slot = u if par == 0 else 2 + u
        return g * 4 * SS + slot * SS

    with (
        tc.tile_pool(name="constp", bufs=1) as constp,
        tc.tile_pool(name="qhtp", bufs=1) as qhtp,
        tc.tile_pool(name="dramp", bufs=1, space="DRAM") as dramp,
    ):
        ones = constp.tile([1, 128], f32)
        nc.gpsimd.memset(ones[:], 1.0)
        zb = constp.tile([128, 1], f32)
        nc.gpsimd.memset(zb[:], 0.0)
        z512 = constp.tile([1, 512], f32)
        nc.gpsimd.memset(z512[:], 0.0)
        bq_sb = constp.tile([128, 8], f32)
        nc.sync.dma_start(bq_sb[:], bq.ap().rearrange("(t p) -> p t", p=128))
        bk_sb = constp.tile([128, 8], f32)
        nc.sync.dma_start(bk_sb[:], bk.ap().rearrange("(t p) -> p t", p=128))
        bv_sb = constp.tile([1, DIM], f32)
        nc.sync.dma_start(bv_sb[:], bv.ap().unsqueeze(0))
        bo_sb = constp.tile([1, DIM], f32)
        nc.sync.dma_start(bo_sb[:], bo.ap().unsqueeze(0))

        aspace = "Local" if fake_ag else "Shared"
        ag_in_k = dramp.tile([DIM, SS], bf16)
        ag_in_v = dramp.tile([DIM, SS], bf16)
        ag_out_k = dramp.tile([NCORES * DIM, SS], bf16, addr_space=aspace)
        ag_out_v = dramp.tile([NCORES * DIM, SS], bf16, addr_space=aspace)

        QhT_sb = qhtp.tile([128, 8 * SS], bf16)
        KhT_c2 = qhtp.tile([128, 8 * SS], bf16)
        Vh_c2 = qhtp.tile([128, 2 * DIM], bf16)

        # ---------------- Phase A: projections of the local slice ----------
        # Engine/ring plan: all bulk loads + V-side staging on the SP HWDGE
        # ring; K-side staging + K readbacks on the Pool (SWDGE) ring so they
        # bypass the big weight loads queued on SP; collectives trigger from
        # Pool but run on the collective cores. ACT does drains/exp only.
        with (
            tc.tile_pool(name="wp", bufs=1) as wp,
            tc.tile_pool(name="inp", bufs=1) as inp,
            tc.tile_pool(name="projp", bufs=1) as projp,
            tc.tile_pool(name="psA", bufs=1, space="PSUM") as psA,
        ):
            def load_w(dram_w, name):
                w_sb = wp.tile([128, 8 * DIM], bf16, name=name)
                src = dram_w.ap().rearrange("(t p) d -> p t d", p=128)
                dst = w_sb[:].rearrange("p (t d) -> p t d", t=8)
                for h in range(2):
                    nc.sync.dma_start(dst[:, 4 * h : 4 * h + 4, :],
                                      src[:, 4 * h : 4 * h + 4, :])
                return w_sb

            def load_x(dram_x, name):
                x_sb = inp.tile([128, 8 * SS], bf16, name=name)
                nc.sync.dma_start(
                    x_sb[:].rearrange("p (t j) -> p t j", t=8),
                    dram_x.ap().rearrange("(t p) j -> p t j", p=128),
                )
                return x_sb

            kT_sb = load_x(kT, "kT_sb")
            WkT_sb = load_w(WkT, "WkT_sb")
            qT_sb = load_x(qT, "qT_sb")
            WqT_sb = load_w(WqT, "WqT_sb")
            vT_sb = load_x(vT, "vT_sb")
            WvT_sb = load_w(WvT, "WvT_sb")

            # K^T projection: KhT_c[d_out, j_local] = Wk @ k_c^T + bk
            KhT_c = KhT_c2
            for mt in range(8):
                kps = psA.tile([128, SS], f32, tag="kqps", bufs=4)
                for kt in range(8):
                    nc.tensor.matmul(
                        kps[:],
                        WkT_sb[:, kt * DIM + mt * 128 : kt * DIM + (mt + 1) * 128],
                        kT_sb[:, kt * SS : (kt + 1) * SS],
                        start=(kt == 0), stop=(kt == 7),
                    )
                nc.scalar.activation(
                    KhT_c[:, mt * SS : (mt + 1) * SS], kps[:],
                    AF.Identity, bias=bk_sb[:, mt : mt + 1], scale=1.0,
                )
            nc.gpsimd.dma_start(
                ag_in_k[:, :].rearrange("(t p) j -> p t j", p=128),
                KhT_c[:].rearrange("p (t j) -> p t j", t=8),
            )
            if fake_ag:
                nc.gpsimd.dma_start(
                    ag_out_k[:, :].rearrange("(c r) j -> c r j", c=NCORES)[0],
                    ag_in_k[:, :])
            else:
                nc.gpsimd.collective_compute(
                    "AllGather", mybir.AluOpType.bypass,
                    replica_groups=[list(range(NCORES))],
                    ins=[ag_in_k[:, :]], outs=[ag_out_k[:, :]],
                )

            # Q^T projection (scale pre-folded into WqT/bq on host)
            for mt in range(8):
                qps = psA.tile([128, SS], f32, tag="kqps", bufs=4)
                for kt in range(8):
                    nc.tensor.matmul(
                        qps[:],
                        WqT_sb[:, kt * DIM + mt * 128 : kt * DIM + (mt + 1) * 128],
                        qT_sb[:, kt * SS : (kt + 1) * SS],
                        start=(kt == 0), stop=(kt == 7),
                    )
                nc.scalar.activation(
                    QhT_sb[:, mt * SS : (mt + 1) * SS], qps[:],
                    AF.Identity, bias=bq_sb[:, mt : mt + 1], scale=1.0,
                )

            # V projection (not transposed): Vh_c[j_local, d_out]
            Vh_c = Vh_c2
            for st in range(2):
                for nh in range(2):
                    vps = psA.tile([128, 512], f32, tag="vps", bufs=2)
                    for kt in range(8):
                        nc.tensor.matmul(
                            vps[:],
                            vT_sb[:, kt * SS + st * 128 : kt * SS + (st + 1) * 128],
                            WvT_sb[:, kt * DIM + nh * 512 : kt * DIM + (nh + 1) * 512],
                            start=(kt == 0), stop=False,
                        )
                    nc.tensor.matmul(
                        vps[:], ones[:, 0:128],
                        bv_sb[:, nh * 512 : (nh + 1) * 512],
                        start=False, stop=True,
                    )
                    nc.scalar.activation(
                        Vh_c[:, st * DIM + nh * 512 : st * DIM + (nh + 1) * 512],
                        vps[:], AF.Copy,
                    )
            nc.sync.dma_start(
                ag_in_v[:, :].rearrange("(a p c) j -> p a (c j)", a=2, p=128),
                Vh_c[:].rearrange("p (a d) -> p a d", a=2),
            )
        # ---------------- Phase B: attention over full K/V ------------------
        with (
            tc.tile_pool(name="kvp", bufs=1) as kvp,
            tc.tile_pool(name="attnp", bufs=2) as attnp,
            tc.tile_pool(name="psB", bufs=1, space="PSUM") as psB,
        ):
            KhT_sb = kvp.tile([128, 8 * SEQ], bf16)
            Vh_sb = kvp.tile([128, 16 * DIM], bf16)
            WoT_sb = kvp.tile([128, 8 * DIM], f32)
            ctx_sb = kvp.tile([128, 8 * SS], f32)

            KhT_v = KhT_sb[:].rearrange("p (t j) -> p t j", t=8)
            Vh_v = Vh_sb[:].rearrange("p (jt d) -> p jt d", jt=16)
            # Per-core ROTATED block order: j-position s holds real block
            # (pid+s) % 8. Position 0 is this core's own block, copied
            # SBUF->SBUF from the projection outputs so the first two j-tiles
            # of the attention pipeline start during the AllGather. The
            # output is invariant to j order (softmax stats are per (j,i),
            # ctx is a sum over j), so no downstream indexing changes.
            pid = nc.partition_id()
            nc.gpsimd.dma_start(
                KhT_v[:, :, 0:SS],
                KhT_c2[:].rearrange("p (t j) -> p t j", t=8),
            )
            nc.sync.dma_start(
                Vh_v[:, 0:2, :],
                Vh_c2[:].rearrange("p (a d) -> p a d", a=2),
            )
            # K readbacks on the Pool/SWDGE ring (bypass SP's load queue);
            # V readbacks on SP (needed later, SP queue is empty by then).
            for s in range(1, NCORES):
                blk = (pid + s) % NCORES
                nc.gpsimd.dma_start(
                    KhT_v[:, :, SS * s : SS * (s + 1)],
                    ag_out_k[bass.ds(blk * DIM, DIM), :].rearrange(
                        "(t p) j -> p t j", p=128),
                )
            # V AllGather is triggered here (after the K readbacks) so its
            # sequencer wait cannot delay them; it only needs ag_in_v.
            if fake_ag:
                nc.sync.dma_start(
                    ag_out_v[:, :].rearrange("(c r) j -> c r j", c=NCORES)[0],
                    ag_in_v[:, :])
            else:
                nc.gpsimd.collective_compute(
                    "AllGather", mybir.AluOpType.bypass,
                    replica_groups=[list(range(NCORES))],
                    ins=[ag_in_v[:, :]], outs=[ag_out_v[:, :]],
                )
            for s in range(1, NCORES):
                blk = (pid + s) % NCORES
                nc.sync.dma_start(
                    Vh_v[:, 2 * s : 2 * s + 2, :],
                    ag_out_v[bass.ds(blk * DIM, DIM), :].rearrange(
                        "(a p c2) j -> p a (c2 j)", a=2, p=128),
                )
            # Wo load overlaps the attention phase
            wo_src = WoT.ap().rearrange("(t p) d -> p t d", p=128)
            wo_dst = WoT_sb[:].rearrange("p (t d) -> p t d", t=8)
            for h in range(2):
                nc.sync.dma_start(wo_dst[:, 4 * h : 4 * h + 4, :],
                                  wo_src[:, 4 * h : 4 * h + 4, :])

            ctx_ps = psB.tile([128, 8 * SS], f32, tag="ctx")
            # One start=True matmul per PSUM bank covering the full bank:
            # initializes the whole zero-region so the 16 interleaved per-head
            # accumulation slices can all use start=False (a start=True per
            # slice would re-mark the bank pending and drop prior slices).
            for b in range(4):
                nc.tensor.matmul(
                    ctx_ps[:, 512 * b : 512 * (b + 1)],
                    z512[:, 0:128], z512[:, 0:512],
                    start=True, stop=False, skip_group_check=True,
                )
            attn_q = []

            def emit_ctx(jt, attn):
                for h in range(16):
                    hp, pr = h // 2, h % 2
                    nc.tensor.matmul(
                        ctx_ps[64 * pr : 64 * pr + 64, hp * SS : (hp + 1) * SS],
                        Vh_sb[:, jt * DIM + h * 64 : jt * DIM + (h + 1) * 64],
                        attn[:, slot_col(h) : slot_col(h) + SS],
                        start=False, stop=(jt == 15 and h >= 12),
                        skip_group_check=True,
                    )

            for jt in range(16):
                e_sb = attnp.tile([128, 16 * SS], bf16, tag="e", bufs=3)
                for g in range(4):
                    sc_ps = psB.tile([128, 4 * SS], f32, tag="sc", bufs=2)
                    for u in range(2):
                        for par in range(2):
                            h = 4 * g + 2 * u + par
                            t = h // 2
                            nc.tensor.matmul(
                                sc_ps[:, (u if par == 0 else 2 + u) * SS :][:, :SS],
                                KhT_sb[64 * par : 64 * par + 64,
                                       t * SEQ + jt * 128 : t * SEQ + (jt + 1) * 128],
                                QhT_sb[64 * par : 64 * par + 64,
                                       t * SS : (t + 1) * SS],
                                start=True, stop=True,
                            )
                    nc.scalar.activation(
                        e_sb[:, g * 4 * SS : (g + 1) * 4 * SS], sc_ps[:],
                        AF.Exp, bias=zb[:],
                    )
                t1 = attnp.tile([128, 8 * SS], bf16, tag="t1", bufs=3)
                nc.gpsimd.tensor_add(t1[:], e_sb[:, 0 : 8 * SS],
                                     e_sb[:, 8 * SS : 16 * SS])
                t2 = attnp.tile([128, 4 * SS], bf16, tag="t2", bufs=3)
                nc.vector.tensor_add(t2[:], t1[:, 0 : 4 * SS], t1[:, 4 * SS : 8 * SS])
                t3 = attnp.tile([128, 2 * SS], bf16, tag="t3", bufs=3)
                nc.vector.tensor_add(t3[:], t2[:, 0 : 2 * SS], t2[:, 2 * SS : 4 * SS])
                Dsum = attnp.tile([128, SS], f32, tag="Dsum")
                nc.vector.tensor_add(Dsum[:], t3[:, 0:SS], t3[:, SS : 2 * SS])
                Rf = attnp.tile([128, SS], f32, tag="Rf")
                nc.vector.reciprocal_approx_fast(Rf[:], Dsum[:])
                Rcp = attnp.tile([128, SS], bf16, tag="Rcp")
                nc.gpsimd.tensor_copy(Rcp[:], Rf[:])
                attn = attnp.tile([128, 16 * SS], bf16, tag="attn", bufs=3)
                nc.vector.tensor_mul(
                    attn[:].rearrange("p (s j) -> p s j", s=16),
                    e_sb[:].rearrange("p (s j) -> p s j", s=16),
                    Rcp[:].unsqueeze(1).broadcast_to([128, 16, SS]),
                )
                attn_q.append((jt, attn))
                # software pipeline: emit ctx matmuls one j-tile behind the
                # scores/softmax chain so PE never waits on the current
                # tile's DVE work
                if len(attn_q) > 2:
                    emit_ctx(*attn_q.pop(0))
            while attn_q:
                emit_ctx(*attn_q.pop(0))
            nc.scalar.activation(ctx_sb[:], ctx_ps[:], AF.Copy)

        # ---------------- Phase C: output projection ------------------------
        with (
            tc.tile_pool(name="outp", bufs=1) as outp,
            tc.tile_pool(name="psO", bufs=1, space="PSUM") as psO,
        ):
            out_sb = outp.tile([128, 2 * DIM], f32)
            for mt in range(2):
                for nh in range(2):
                    ops = psO.tile([128, 512], f32, tag="ops", bufs=4)
                    for kt in range(8):
                        nc.tensor.matmul(
                            ops[:],
                            ctx_sb[:, kt * SS + mt * 128 : kt * SS + (mt + 1) * 128],
                            WoT_sb[:, kt * DIM + nh * 512 : kt * DIM + (nh + 1) * 512],
                            start=(kt == 0), stop=False,
                        )
                    nc.tensor.matmul(
                        ops[:], ones[:, 0:128],
                        bo_sb[:, nh * 512 : (nh + 1) * 512],
                        start=False, stop=True,
                    )
                    nc.scalar.activation(
                        out_sb[:, mt * DIM + nh * 512 : mt * DIM + (nh + 1) * 512],
                        ops[:], AF.Copy,
                    )
                nc.sync.dma_start(
                    out.ap().rearrange("(mt p) d -> p mt d", p=128)[:, mt, :],
                    out_sb[:, mt * DIM : (mt + 1) * DIM],
                )


def get_nc():
    if "nc" not in _CACHE:
        _CACHE["nc"] = _build()
    return _CACHE["nc"]


def make_in_maps(inputs):
    f = lambda x: np.ascontiguousarray(np.asarray(x, dtype=np.float32))
    bf = ml_dtypes.bfloat16
    q, k, v = f(inputs["q"]), f(inputs["k"]), f(inputs["v"])
    WqTs = np.ascontiguousarray((f(inputs["Wq"]) * SCALE).T.astype(bf))
    WkT = np.ascontiguousarray(f(inputs["Wk"]).T.astype(bf))
    WvT = np.ascontiguousarray(f(inputs["Wv"]).T.astype(bf))
    WoT = np.ascontiguousarray(f(inputs["Wo"]).T)
    bqs = f(inputs["bq"]) * np.float32(SCALE)
    bk, bv, bo = f(inputs["bk"]), f(inputs["bv"]), f(inputs["bo"])
    in_maps = []
    for c in range(NCORES):
        sl = slice(c * SS, (c + 1) * SS)
        in_maps.append({
            "qT": np.ascontiguousarray(q[sl].T.astype(bf)),
            "kT": np.ascontiguousarray(k[sl].T.astype(bf)),
            "vT": np.ascontiguousarray(v[sl].T.astype(bf)),
            "WqT": WqTs, "WkT": WkT, "WvT": WvT, "WoT": WoT,
            "bq": bqs, "bk": bk, "bv": bv, "bo": bo,
        })
    return in_maps


def run(inputs, **kwargs):
    """Run on hardware; returns (output, BassKernelResults)."""
    from concourse import bass_utils

    nc = get_nc()
    res = bass_utils.run_bass_kernel_spmd(
        nc, make_in_maps(inputs), core_ids=list(range(NCORES)), **kwargs
    )
    rows = [res.results[c]["out"] for c in range(NCORES)]
    full = np.concatenate(rows, axis=0).astype(np.float32)
    return full.reshape(1, SEQ, DIM), res


def kernel(**inputs) -> np.ndarray:
    out, _ = run(inputs)
    return out
